# revision 1
# baseline (speedup 1.0000x reference)
"""Negative cross-correlation loss: out = -sum(x * y).

Full inputs x, y: (16, 4000, 512, 1) f32. Data-parallel over the shot axis:
2 shots per core on 8 NeuronCores. Each core DMAs its 2x4000x512 shard as
8 tiles of [128, 4000], fuses multiply+per-partition-reduce on the vector
engine (scalar_tensor_tensor accum_out), reduces across tiles and
partitions, and writes one scalar. Host sums the 8 partials and negates.
"""

import numpy as np

import jax
from jax.experimental.shard_map import shard_map
from jax.sharding import Mesh, NamedSharding, PartitionSpec

import concourse.bacc as bacc
import concourse.mybir as mybir
import concourse.tile as tile
from concourse import bass2jax
from concourse.bass_isa import ReduceOp

N_CORES = 8
P = 128
# Per-core shard: 2 shots * 4000 * 512 * 1 = 4_096_000 f32 elements.
SHARD_ELEMS = 2 * 4000 * 512
TILE_W = 4000
N_TILES = SHARD_ELEMS // (P * TILE_W)  # 8
assert N_TILES * P * TILE_W == SHARD_ELEMS


def _build_nc(
    repeat=1,
    tile_w=TILE_W,
    bufs=6,
    skip_compute=False,
    barrier_between=False,
    host_reduce=True,
    split_dma=False,
    taper=(1500, 1000, 750, 500, 250),
    out_via_act=False,
    split_out=0,
):
    """Bass kernel for one core. `repeat` re-runs the identical body that many
    times (same data, same result) — used only for wall-clock slope timing."""
    n_tiles = SHARD_ELEMS // (P * tile_w)
    assert n_tiles * P * tile_w == SHARD_ELEMS
    # Tile list: (row_block, col_offset, width). `taper` replaces the last
    # full-width tile with narrower ones so the final DVE op (which can only
    # start after the last DMA lands) is short — shrinks the kernel tail.
    tiles = [(t, 0, tile_w) for t in range(n_tiles)]
    if taper:
        assert sum(taper) == tile_w
        last = tiles.pop()[0]
        off = 0
        for w in taper:
            tiles.append((last, off, w))
            off += w
    nc = bacc.Bacc("TRN2", target_bir_lowering=False, debug=False)
    x = nc.dram_tensor("x", [N_TILES * P, TILE_W], mybir.dt.float32, kind="ExternalInput")
    y = nc.dram_tensor("y", [N_TILES * P, TILE_W], mybir.dt.float32, kind="ExternalInput")
    out_shape = [P, len(tiles)] if host_reduce else [1, 1]
    out = nc.dram_tensor("out", out_shape, mybir.dt.float32, kind="ExternalOutput")

    # The DRAM I/O shape is fixed at [N_TILES*P, TILE_W]; re-view it at the
    # requested tile width (pure elementwise reduction — layout-agnostic).
    def _view(ap):
        if tile_w > TILE_W:
            return ap.rearrange("(r s) c -> r (s c)", s=tile_w // TILE_W)
        if tile_w < TILE_W:
            return ap.rearrange("r (s c) -> (r s) c", c=tile_w)
        return ap

    xa = _view(x.ap())
    ya = _view(y.ap())
    oa = out.ap()

    with tile.TileContext(nc) as tc:
        with (
            tc.tile_pool(name="io", bufs=bufs) as io_pool,
            tc.tile_pool(name="red", bufs=1) as red_pool,
        ):
            acc = red_pool.tile([P, len(tiles)], mybir.dt.float32)
            dummy = red_pool.tile([P, 1], mybir.dt.float32)
            if skip_compute:
                nc.vector.memset(acc[:], 0.0)
            for rep in range(repeat):
                if barrier_between and rep:
                    # serialize repeats: each runs from a drained pipeline,
                    # so slope over `repeat` ~ true single-shot body time
                    tc.strict_bb_all_engine_barrier()
                for i, (t, off, w) in enumerate(tiles):
                    xt = io_pool.tile([P, tile_w], mybir.dt.float32, tag="xt")
                    yt = io_pool.tile([P, tile_w], mybir.dt.float32, tag="yt")
                    # x on the SP HWDGE ring, y on the otherwise-idle ACT
                    # ring: two descriptor streams feed the SDMA engines.
                    y_dma = nc.scalar if split_dma else nc.sync
                    rows = slice(t * P, (t + 1) * P)
                    cols = slice(off, off + w)
                    nc.sync.dma_start(out=xt[:, :w], in_=xa[rows, cols])
                    y_dma.dma_start(out=yt[:, :w], in_=ya[rows, cols])
                    if skip_compute:  # timing diagnostic only: wrong output
                        continue
                    # acc[:, i] = sum_w xt*yt (per-partition); dummy absorbs
                    # the elementwise product via a stride-0 output.
                    nc.vector.scalar_tensor_tensor(
                        out=dummy.broadcast_to(xt[:, :w].shape),
                        in0=xt[:, :w],
                        scalar=1.0,
                        in1=yt[:, :w],
                        op0=mybir.AluOpType.mult,
                        op1=mybir.AluOpType.mult,
                        accum_out=acc[:, i : i + 1],
                    )
            if host_reduce:
                # ship the [P, n_tiles] partials (4 KB); host finishes the sum
                out_engine = nc.scalar if out_via_act else nc.sync
                if split_out:
                    # early chunk hides under the input stream; only the last
                    # columns ride the final STT->DMA dependency chain
                    k = len(tiles) - split_out
                    out_engine.dma_start(out=oa[:, :k], in_=acc[:, :k])
                    out_engine.dma_start(out=oa[:, k:], in_=acc[:, k:])
                else:
                    out_engine.dma_start(out=oa[:, :], in_=acc[:])
            else:
                total = red_pool.tile([P, 1], mybir.dt.float32)
                nc.vector.tensor_reduce(
                    out=total[:],
                    in_=acc[:],
                    axis=mybir.AxisListType.X,
                    op=mybir.AluOpType.add,
                )
                nc.gpsimd.partition_all_reduce(total[:], total[:], P, ReduceOp.add)
                nc.sync.dma_start(out=oa[:, :], in_=total[:1, :1])

    nc.compile()
    return nc


class Runner:
    """Compiles the per-core Bass kernel once and keeps a cached jitted
    shard_map executable over 8 cores (mirrors bass2jax.run_bass_via_pjrt's
    multi-core path, minus the per-call retrace and host concat)."""

    def __init__(self, repeat=1, n_chained=1, **build_kwargs):
        bass2jax.install_neuronx_cc_hook()
        nc = _build_nc(repeat, **build_kwargs)
        self.nc = nc

        in_names = ["x", "y"]
        out_names = ["out"]
        out_shape = None
        for alloc in nc.m.functions[0].allocations:
            if (
                isinstance(alloc, mybir.MemoryLocationSet)
                and alloc.kind == "ExternalOutput"
            ):
                out_shape = tuple(alloc.tensor_shape)
        assert out_shape is not None
        self.out_shape = out_shape
        out_avals = (jax.core.ShapedArray(out_shape, np.float32),)
        all_in_names = tuple(in_names + out_names + [nc.partition_id_tensor.name])

        def _body(x, y, z):
            # n_chained > 1 (timing only): run the same NEFF k times back to
            # back, threading each exec's output in as the next one's
            # out-buffer operand so the execs can't be deduped or reordered.
            # The slope of wall time over k is the full per-NEFF exec time.
            pid = bass2jax.partition_id_tensor()
            o = z
            for _ in range(n_chained):
                (o,) = bass2jax._bass_exec_p.bind(
                    x,
                    y,
                    o,
                    pid,
                    out_avals=out_avals,
                    in_names=all_in_names,
                    out_names=tuple(out_names),
                    lowering_input_output_aliases=(),
                    sim_require_finite=True,
                    sim_require_nnan=True,
                    nc=nc,
                )
            return (o,)

        devices = jax.devices()[:N_CORES]
        assert len(devices) == N_CORES
        self.mesh = Mesh(np.asarray(devices), ("core",))
        self.sharding = NamedSharding(self.mesh, PartitionSpec("core"))
        in_specs = (PartitionSpec("core"),) * 3
        out_specs = (PartitionSpec("core"),)
        self.fn = jax.jit(
            shard_map(
                _body,
                mesh=self.mesh,
                in_specs=in_specs,
                out_specs=out_specs,
                check_rep=False,
            ),
            donate_argnums=(2,),
            keep_unused=True,
        )

    def __call__(self, x_all, y_all):
        """x_all, y_all: [N_CORES * N_TILES * P, TILE_W] f32 (host or device).
        Returns the per-core partial sums, one row per core."""
        zeros = np.zeros((N_CORES * self.out_shape[0], *self.out_shape[1:]), np.float32)
        (out,) = self.fn(x_all, y_all, zeros)
        return np.asarray(out).reshape(N_CORES, -1).sum(axis=1, dtype=np.float64)


_RUNNER = None


def _get_runner():
    global _RUNNER
    if _RUNNER is None:
        _RUNNER = Runner()
    return _RUNNER


def _run_via_spmd(x, y):
    """Fallback for non-axon containers (real /dev/neuron*): the library's own
    SPMD entrypoint, which picks the native-NRT or PJRT path as appropriate."""
    from concourse.bass_utils import run_bass_kernel_spmd

    rows = N_TILES * P
    nc = _build_nc()
    in_maps = [
        {
            "x": np.ascontiguousarray(x[c * rows : (c + 1) * rows]),
            "y": np.ascontiguousarray(y[c * rows : (c + 1) * rows]),
        }
        for c in range(N_CORES)
    ]
    res = run_bass_kernel_spmd(nc, in_maps, core_ids=list(range(N_CORES)))
    return np.array([np.float64(r["out"].sum()) for r in res.results])


def kernel(x, y, win=None, step=None):
    # Row-block c of the reshaped [8192, TILE_W] array is exactly core c's
    # shard (shots 2c, 2c+1) — shard_map's axis-0 split does the sharding.
    x = np.ascontiguousarray(np.asarray(x, dtype=np.float32)).reshape(
        N_CORES * N_TILES * P, TILE_W
    )
    y = np.ascontiguousarray(np.asarray(y, dtype=np.float32)).reshape(
        N_CORES * N_TILES * P, TILE_W
    )
    try:
        parts = _get_runner()(x, y)
    except Exception:
        parts = _run_via_spmd(x, y)
    return np.float32(-np.float64(parts.sum()))

